# revision 49
# baseline (speedup 1.0000x reference)
"""Neural CDE forward pass on 8 Trainium2 NeuronCores (pure data parallel).

B=512 batch is sharded 64 per core.  Per core, the entire 30-step RK4
integration (120 vector-field evals) runs out of SBUF:

  - small MLP layers feature-major: lhsT = weight chunk (stationary),
    rhs = activation [feat, 64]; bias+relu on Pool (A half) and DVE
    (B half) so ACT stays free for tanh.
  - W_out layer batch-major, split-h: y PSUM [128 = 2 h-halves x 64 batch,
    cols = (h_local, i_pad)] so tanh/mul/reduce use all 128 lanes.
    b_out enters via a ones-row appended to the stationary activation.
  - einsum g[b,h] = sum_i tanh(y)[b,h,i] * dX[b,i]: DVE multiply with a
    broadcast dX tile (fp16) + segmented fp16 tensor_reduce over i.
  - k is transposed to feature-major per region-group with small PE
    matmuls through a stacked identity, so the z state stays
    feature-major and never needs a post-update transpose or cast.
  - dX for the 75 distinct (interval, s) points is computed on host and
    DMA'd once.
"""

import os
import numpy as np

B, T, IN, HID, HH, NCLS = 512, 16, 41, 64, 150, 4
N_SUB, NL = 2, 3
NCORES = 8
BL = B // NCORES            # 64 per-core batch
INP = IN + 1                # 42: i padded for even segments
NIV = T - 1                 # 15 intervals
NS = 5                      # distinct s values per interval
HSTEP = 1.0 / N_SUB         # 0.5
HALF_COLS = 32 * INP        # 1344 cols per h-half
# (h0, hcount, bank, bank_col) splits of the 32 h per half; each region's
# cols (hcount*42) must fit one 2KB PSUM bank (<=512 fp32) since a matmul
# cannot cross banks.  First region small so the tanh/mul/reduce chain
# starts early; last region tiny so the final reduce drains fast.  Regions
# 0 and 3 share PSUM bank 0 at different column offsets.
REGIONS = [(0, 12, 0, 0), (12, 10, 1, 0), (22, 10, 2, 0)]
# k-transpose groups (h0, hc): matmul out base partition must be 0/32/64,
# so a single [0,32) group (bases 0 and 32) is the only legal split
KT_GROUPS = [(0, 32, 2)]  # (h0, hc, after_region_idx)
STEPS = int(os.environ.get("NCDE_STEPS", NIV * N_SUB))  # debug knob


def _prep_shared(W0, b0, W_in, b_in, W_h, b_h, W_out, b_out, Wc1, bc1, Wc2, bc2):
    f16 = np.float16
    f32 = np.float32
    wha = np.concatenate([W_h[i][0:128, :] for i in range(NL)], axis=1)
    whb = np.concatenate([W_h[i][128:HH, :] for i in range(NL)], axis=1)
    bias_a = np.stack([b_in[0:128]] + [b_h[i][0:128] for i in range(NL)], axis=1)
    bias_b = np.stack([b_in[128:HH]] + [b_h[i][128:HH] for i in range(NL)], axis=1)
    R = W_out.reshape(HH, HID, IN)
    Rp = np.zeros((HH, HID, INP), np.float32)
    Rp[:, :, :IN] = R
    W2 = np.concatenate(
        [Rp[:, 0:32, :].reshape(HH, HALF_COLS), Rp[:, 32:64, :].reshape(HH, HALF_COLS)],
        axis=1,
    )
    bo = np.zeros((HID, INP), np.float32)
    bo[:, :IN] = b_out.reshape(HID, IN)
    bo2 = np.concatenate([bo[0:32].reshape(-1), bo[32:64].reshape(-1)])
    i64 = np.eye(64, dtype=np.float32)
    return {
        "w0": W0.astype(f16),
        "b0c": b0.reshape(HID, 1).astype(f32),
        "wi": W_in.astype(f16),
        "wha": wha.astype(f16),
        "whb": whb.astype(f16),
        "bias_a": bias_a.astype(f32),
        "bias_b": bias_b.astype(f32),
        "woa": W2[0:128].astype(f16),
        "wob": np.vstack([W2[128:HH], bo2[None]]).astype(f16),
        "wc1": Wc1.astype(f16),
        "bc1c": bc1.reshape(HID, 1).astype(f32),
        "wc2": Wc2.astype(f16),
        "bc2c": bc2.reshape(NCLS, 1).astype(f32),
        "eperm": np.vstack(
            [np.hstack([i64, 0 * i64]), np.hstack([0 * i64, i64])]
        ).astype(f16),
        "hhB_init": np.vstack(
            [np.zeros((HH - 128, BL), np.float32), np.ones((1, BL), np.float32)]
        ).astype(f16),
    }


def _prep_percore(bc_core):
    """bc_core: [BL, NIV, 4, IN] fp32 -> x0t [IN, BL] f16, dxh [128, NS*NIV*INP] f16."""
    x0t = bc_core[:, 0, 0, :].T.astype(np.float16)
    c1 = bc_core[:, :, 1, :]  # [BL, NIV, IN]
    c2 = bc_core[:, :, 2, :]
    c3 = bc_core[:, :, 3, :]
    dxh = np.zeros((128, NIV * NS * INP), np.float32)
    for iv in range(NIV):
        for si in range(NS):
            s = si * 0.25
            dX = c1[:, iv] + (2.0 * s) * c2[:, iv] + (3.0 * s * s) * c3[:, iv]
            col = (iv * NS + si) * INP
            dxh[0:BL, col : col + IN] = dX
            dxh[BL:128, col : col + IN] = dX
    return x0t, dxh.astype(np.float16)


def build_nc(steps=STEPS):
    """Build the single-core Bass program (same program on all 8 cores)."""
    from contextlib import ExitStack

    import concourse.bass as bass
    import concourse.mybir as mybir
    from concourse import bacc, tile

    f16 = mybir.dt.float16
    f32 = mybir.dt.float32
    AF = mybir.ActivationFunctionType
    OP = mybir.AluOpType

    nc = bacc.Bacc("TRN2", target_bir_lowering=False, debug=False)

    dram = {}
    ins_spec = [
        ("x0t", [IN, BL], f16),
        ("dxh", [128, NIV * NS * INP], f16),
        ("w0", [IN, HID], f16),
        ("b0c", [HID, 1], f32),
        ("wi", [HID, HH], f16),
        ("wha", [128, NL * HH], f16),
        ("whb", [HH - 128, NL * HH], f16),
        ("bias_a", [128, 1 + NL], f32),
        ("bias_b", [HH - 128, 1 + NL], f32),
        ("woa", [128, 2 * HALF_COLS], f16),
        ("wob", [HH - 128 + 1, 2 * HALF_COLS], f16),
        ("wc1", [HID, HID], f16),
        ("bc1c", [HID, 1], f32),
        ("wc2", [HID, NCLS], f16),
        ("bc2c", [NCLS, 1], f32),
        ("eperm", [128, 128], f16),
        ("hhB_init", [HH - 128 + 1, BL], f16),
    ]
    for name, shape, dt in ins_spec:
        dram[name] = nc.dram_tensor(name, shape, dt, kind="ExternalInput")
    out_dram = nc.dram_tensor("pred_t", [NCLS, BL], f32, kind="ExternalOutput")

    with tile.TileContext(nc) as tc:
        with ExitStack() as ctx:
            const = ctx.enter_context(tc.tile_pool(name="const", bufs=1))
            work = ctx.enter_context(tc.tile_pool(name="work", bufs=3))
            ty_pool = ctx.enter_context(tc.tile_pool(name="ty", bufs=3))
            pr_pool = ctx.enter_context(tc.tile_pool(name="pr", bufs=3))
            kt_pool = ctx.enter_context(tc.tile_pool(name="kt", bufs=2))
            ps_h = ctx.enter_context(
                tc.tile_pool(name="ps_h", bufs=2, space=bass.MemorySpace.PSUM)
            )
            ps_hb = ctx.enter_context(
                tc.tile_pool(name="ps_hb", bufs=2, space=bass.MemorySpace.PSUM)
            )
            ps_y = ctx.enter_context(
                tc.tile_pool(name="ps_y", bufs=1, space=bass.MemorySpace.PSUM)
            )
            ps_k = ctx.enter_context(
                tc.tile_pool(name="ps_k", bufs=1, space=bass.MemorySpace.PSUM)
            )

            # ---- load constants/weights into SBUF --------------------------
            # two HWDGE queues (SP + ACT) in parallel, earliest-needed
            # tensors first on each; big einsum/W_out tensors next,
            # classifier last
            dma_sp = ["w0", "x0t", "b0c", "wi", "bias_a", "eperm", "woa",
                      "dxh", "wc1", "bc1c"]
            dma_act = ["wha", "whb", "bias_b", "hhB_init", "wob",
                       "wc2", "bc2c"]
            shapes = {name: (shape, dt) for name, shape, dt in ins_spec}
            sb = {}
            for names, eng in ((dma_sp, nc.sync), (dma_act, nc.scalar)):
                for name in names:
                    shape, dt = shapes[name]
                    t = const.tile(shape, dt, tag=name)
                    eng.dma_start(t[:], dram[name][:])
                    sb[name] = t

            # persistent state tiles (hhB arrives with its ones row preset)
            hhB = sb["hhB_init"]
            zfmA = const.tile([HID, BL], f32, tag="zfmA")   # master z (feature-major)
            zfmB = const.tile([HID, BL], f32, tag="zfmB")
            zacc = const.tile([HID, BL], f32, tag="zacc")

            # psum y region tiles (persistent; serial stages reuse them)
            # one full 2KB bank each so every tile starts bank-aligned
            yR = [
                ps_y.tile([128, 512], f32, tag=f"yR{rt}", name=f"yR{rt}")
                for rt in range(3)
            ]

            # ---- initial state z0 = X0 @ W0 + b0 (feature-major) -----------
            z0p = ps_h.tile([HID, BL], f32, tag="hA")
            nc.tensor.matmul(z0p[:], sb["w0"][:], sb["x0t"][:])
            zT = work.tile([HID, BL], f16, tag="zT")
            nc.vector.tensor_scalar(zT[:], z0p[:], sb["b0c"][:], None, OP.add)
            zfm = zfmA
            zfm_nxt = zfmB
            nc.vector.tensor_scalar(zfm[:], z0p[:], sb["b0c"][:], None, OP.add)

            # RK4 coefficients
            acc_w = [HSTEP / 6.0, HSTEP / 3.0, HSTEP / 3.0, HSTEP / 6.0]
            inp_w = [0.5 * HSTEP, 0.5 * HSTEP, HSTEP, None]

            # off-critical-path accumulator updates are deferred into the
            # next stage's MLP phase so they never delay the zT handoff
            pending_acc = []

            def flush_acc():
                while pending_acc:
                    out_t, in0, scal, in1 = pending_acc.pop(0)
                    nc.vector.scalar_tensor_tensor(
                        out_t[:], in0[:], scal, in1[:], OP.mult, OP.add
                    )

            # ---- time stepping --------------------------------------------
            for step in range(steps):
                iv, sub = step // N_SUB, step % N_SUB
                for stg in range(4):
                    sidx = 2 * sub + (0 if stg == 0 else (1 if stg < 3 else 2))
                    dxcol = (iv * NS + sidx) * INP

                    # -- small MLP: W_in then NL hidden layers (feature-major)
                    hA = None
                    hB = None
                    for layer in range(1 + NL):
                        if layer == 0:
                            wa_l = sb["wi"][:]
                            wb_l = None
                        else:
                            c0 = (layer - 1) * HH
                            wa_l = sb["wha"][:, c0 : c0 + HH]
                            wb_l = sb["whb"][:, c0 : c0 + HH]
                        pA = ps_h.tile([128, BL], f32, tag="hA")
                        pB = ps_hb.tile([HH - 128, BL], f32, tag="hB")
                        if layer == 0:
                            nc.tensor.matmul(pA[:], wa_l[:, 0:128], zT[:])
                            nc.tensor.matmul(pB[:], wa_l[:, 128:HH], zT[:])
                        else:
                            # pA pair first: its relu gates the next layer,
                            # while the pB pair streams during that relu
                            nc.tensor.matmul(
                                pA[:], wa_l[:, 0:128], hA[:], start=True, stop=False
                            )
                            nc.tensor.matmul(
                                pA[:], wb_l[:, 0:128], hB[:], start=False, stop=True
                            )
                            nc.tensor.matmul(
                                pB[:], wa_l[:, 128:HH], hA[:], start=True, stop=False
                            )
                            nc.tensor.matmul(
                                pB[:], wb_l[:, 128:HH], hB[:], start=False, stop=True
                            )
                        last = layer == NL
                        nhA = work.tile([128, BL], f16, tag="hA_sb")
                        nhB = hhB[0 : HH - 128, :] if last else work.tile(
                            [HH - 128, BL], f16, tag="hB_sb"
                        )
                        ba = sb["bias_a"][:, layer : layer + 1]
                        bb = sb["bias_b"][:, layer : layer + 1]
                        # A on DVE (lower PSUM->SBUF latency than ACT), B on
                        # ACT: the two bias+relu ops run concurrently
                        nc.vector.tensor_scalar(nhA[:], pA[:], ba, 0.0, OP.add, OP.max)
                        nc.scalar.activation(nhB[:], pB[:], AF.Relu, bias=bb)
                        hA, hB = nhA, (hhB[0 : HH - 128 + 1, :] if last else nhB)
                        if layer == 0:
                            flush_acc()

                    # -- W_out: y[p = half*64+b, (h_local, i)]  (batch-major)
                    # region-major so each region completes early and the
                    # tanh/einsum chain starts while later regions stream
                    for rt, (h0, hc, bk, bc) in enumerate(REGIONS):
                        for kc in range(2):
                            lhs = hA[:] if kc == 0 else hhB[:]
                            rhs_t = sb["woa"] if kc == 0 else sb["wob"]
                            for half in range(2):
                                cols = half * HALF_COLS + h0 * INP
                                # lo/hi halves accumulate in disjoint
                                # partition rows of one bank; the sim's group
                                # guard is partition-blind, so skip it.
                                nc.tensor.matmul(
                                    yR[bk][half * 64 : half * 64 + 64, bc : bc + hc * INP],
                                    lhs,
                                    rhs_t[:, cols : cols + hc * INP],
                                    start=(kc == 0),
                                    stop=(kc == 1),
                                    skip_group_check=True,
                                )

                    # -- tanh -> multiply by dX -> segmented reduce over i
                    # (pad col 41 skipped on DVE; k_t fp16 for 2x multiply)
                    k_t = kt_pool.tile([128, 32], f16, tag="k")
                    for rt, (h0, hc, bk, bc) in enumerate(REGIONS):
                        # tanh skips the pad column via a 3D strided read
                        ty = ty_pool.tile([128, hc * IN], f16, tag=f"ty{rt}")
                        yv = (
                            yR[bk][:, bc : bc + hc * INP]
                            .rearrange("p (h i) -> p h i", i=INP)[:, :, 0:IN]
                        )
                        tyv = ty[:].rearrange("p (h i) -> p h i", i=IN)
                        nc.scalar.activation(tyv, yv, AF.Tanh)
                        pr = pr_pool.tile([128, hc * IN], f16, tag=f"pr{rt}")
                        dxv = (
                            sb["dxh"][:, dxcol : dxcol + IN]
                            .unsqueeze(1)
                            .broadcast_to((128, hc, IN))
                        )
                        prv = pr[:].rearrange("p (h i) -> p h i", i=IN)
                        nc.vector.tensor_tensor(prv, tyv, dxv, OP.mult)
                        with nc.allow_low_precision(reason="k reduce fp16 ok"):
                            nc.vector.tensor_reduce(
                                k_t[:, h0 : h0 + hc], prv, mybir.AxisListType.X, OP.add
                            )

                    # -- transpose k to feature-major in region groups so the
                    # z update is a single Pool op with no cast afterwards
                    kfm = ps_k.tile([HID, BL], f32, tag="kfm")
                    for h0, hc, _ in KT_GROUPS:
                        nc.tensor.matmul(
                            kfm[h0 : h0 + hc, :],
                            k_t[:, h0 : h0 + hc],
                            sb["eperm"][:, 0:64],
                            skip_group_check=True,
                        )
                        nc.tensor.matmul(
                            kfm[32 + h0 : 32 + h0 + hc, :],
                            k_t[:, h0 : h0 + hc],
                            sb["eperm"][:, 64:128],
                            skip_group_check=True,
                        )

                    # -- z updates, all feature-major.  zT (gates next MLP)
                    # now; the accumulator update is deferred into the next
                    # stage's MLP phase (flush_acc)
                    zT = work.tile([HID, BL], f16, tag="zT")
                    if stg < 3:
                        nc.vector.scalar_tensor_tensor(
                            zT[:], kfm[:], inp_w[stg], zfm[:], OP.mult, OP.add
                        )
                        pending_acc.append(
                            (zacc, kfm, acc_w[stg], zfm if stg == 0 else zacc)
                        )
                    else:
                        nc.vector.scalar_tensor_tensor(
                            zT[:], kfm[:], acc_w[3], zacc[:], OP.mult, OP.add
                        )
                        pending_acc.append((zfm_nxt, kfm, acc_w[3], zacc))
                        zfm, zfm_nxt = zfm_nxt, zfm

            # ---- classifier on final state --------------------------------
            pending_acc.clear()  # final master-z write is never read
            c1p = ps_h.tile([HID, BL], f32, tag="hA")
            nc.tensor.matmul(c1p[:], sb["wc1"][:], zT[:])
            c1 = work.tile([HID, BL], f16, tag="c1")
            nc.vector.tensor_scalar(c1[:], c1p[:], sb["bc1c"][:], 0.0, OP.add, OP.max)
            c2p = ps_hb.tile([NCLS, BL], f32, tag="hB")
            nc.tensor.matmul(c2p[:], sb["wc2"][:], c1[:])
            pred = work.tile([NCLS, BL], f32, tag="pred")
            nc.vector.tensor_scalar(pred[:], c2p[:], sb["bc2c"][:], None, OP.add)
            nc.sync.dma_start(out_dram[:], pred[:])

    nc.compile()
    return nc


def make_in_maps(inputs):
    shared = _prep_shared(
        inputs["W0"], inputs["b0"], inputs["W_in"], inputs["b_in"],
        inputs["W_h"], inputs["b_h"], inputs["W_out"], inputs["b_out"],
        inputs["Wc1"], inputs["bc1"], inputs["Wc2"], inputs["bc2"],
    )
    bc = np.asarray(inputs["batch_coeffs"], np.float32)
    in_maps = []
    for c in range(NCORES):
        x0t, dxh = _prep_percore(bc[c * BL : (c + 1) * BL])
        in_maps.append({**shared, "x0t": x0t, "dxh": dxh})
    return in_maps


_CACHED = {}


def kernel(**inputs):
    from concourse.bass_utils import run_bass_kernel_spmd

    if "nc" not in _CACHED:
        _CACHED["nc"] = build_nc()
    nc = _CACHED["nc"]
    in_maps = make_in_maps(inputs)
    res = run_bass_kernel_spmd(
        nc, in_maps, core_ids=list(range(NCORES)),
        trace=bool(int(os.environ.get("NCDE_TRACE", "0"))),
    )
    _CACHED["last_result"] = res
    out = np.zeros((B, NCLS), np.float32)
    for c in range(NCORES):
        out[c * BL : (c + 1) * BL, :] = res.results[c]["pred_t"].T
    return out


# revision 51
# speedup vs baseline: 1.0277x; 1.0277x over previous
"""Neural CDE forward pass on 8 Trainium2 NeuronCores (pure data parallel).

B=512 batch is sharded 64 per core.  Per core, the entire 30-step RK4
integration (120 vector-field evals) runs out of SBUF:

  - small MLP layers feature-major: lhsT = weight chunk (stationary),
    rhs = activation [feat, 64]; bias+relu on Pool (A half) and DVE
    (B half) so ACT stays free for tanh.
  - W_out layer batch-major, split-h: y PSUM [128 = 2 h-halves x 64 batch,
    cols = (h_local, i_pad)] so tanh/mul/reduce use all 128 lanes.
    b_out enters via a ones-row appended to the stationary activation.
  - einsum g[b,h] = sum_i tanh(y)[b,h,i] * dX[b,i]: DVE multiply with a
    broadcast dX tile (fp16) + segmented fp16 tensor_reduce over i.
  - k is transposed to feature-major per region-group with small PE
    matmuls through a stacked identity, so the z state stays
    feature-major and never needs a post-update transpose or cast.
  - dX for the 75 distinct (interval, s) points is computed on host and
    DMA'd once.
"""

import os
import numpy as np

B, T, IN, HID, HH, NCLS = 512, 16, 41, 64, 150, 4
N_SUB, NL = 2, 3
NCORES = 8
BL = B // NCORES            # 64 per-core batch
INP = IN + 1                # 42: i padded for even segments
NIV = T - 1                 # 15 intervals
NS = 5                      # distinct s values per interval
HSTEP = 1.0 / N_SUB         # 0.5
HALF_COLS = 32 * INP        # 1344 cols per h-half
# (h0, hcount, bank, bank_col) splits of the 32 h per half; each region's
# cols (hcount*42) must fit one 2KB PSUM bank (<=512 fp32) since a matmul
# cannot cross banks.  First region small so the tanh/mul/reduce chain
# starts early; last region tiny so the final reduce drains fast.  Regions
# 0 and 3 share PSUM bank 0 at different column offsets.
REGIONS = [(0, 11, 0, 0), (11, 10, 1, 0), (21, 11, 2, 0)]
# k-transpose groups (h0, hc): matmul out base partition must be 0/32/64,
# so a single [0,32) group (bases 0 and 32) is the only legal split
KT_GROUPS = [(0, 32, 2)]  # (h0, hc, after_region_idx)
STEPS = int(os.environ.get("NCDE_STEPS", NIV * N_SUB))  # debug knob


def _prep_shared(W0, b0, W_in, b_in, W_h, b_h, W_out, b_out, Wc1, bc1, Wc2, bc2):
    f16 = np.float16
    f32 = np.float32
    wha = np.concatenate([W_h[i][0:128, :] for i in range(NL)], axis=1)
    whb = np.concatenate([W_h[i][128:HH, :] for i in range(NL)], axis=1)
    bias_a = np.stack([b_in[0:128]] + [b_h[i][0:128] for i in range(NL)], axis=1)
    bias_b = np.stack([b_in[128:HH]] + [b_h[i][128:HH] for i in range(NL)], axis=1)
    R = W_out.reshape(HH, HID, IN)
    Rp = np.zeros((HH, HID, INP), np.float32)
    Rp[:, :, :IN] = R
    W2 = np.concatenate(
        [Rp[:, 0:32, :].reshape(HH, HALF_COLS), Rp[:, 32:64, :].reshape(HH, HALF_COLS)],
        axis=1,
    )
    bo = np.zeros((HID, INP), np.float32)
    bo[:, :IN] = b_out.reshape(HID, IN)
    bo2 = np.concatenate([bo[0:32].reshape(-1), bo[32:64].reshape(-1)])
    i64 = np.eye(64, dtype=np.float32)
    return {
        "w0": W0.astype(f16),
        "b0c": b0.reshape(HID, 1).astype(f32),
        "wi": W_in.astype(f16),
        "wha": wha.astype(f16),
        "whb": whb.astype(f16),
        "bias_a": bias_a.astype(f32),
        "bias_b": bias_b.astype(f32),
        "woa": W2[0:128].astype(f16),
        "wob": np.vstack([W2[128:HH], bo2[None]]).astype(f16),
        "wc1": Wc1.astype(f16),
        "bc1c": bc1.reshape(HID, 1).astype(f32),
        "wc2": Wc2.astype(f16),
        "bc2c": bc2.reshape(NCLS, 1).astype(f32),
        "eperm": np.vstack(
            [np.hstack([i64, 0 * i64]), np.hstack([0 * i64, i64])]
        ).astype(f16),
        "hhB_init": np.vstack(
            [np.zeros((HH - 128, BL), np.float32), np.ones((1, BL), np.float32)]
        ).astype(f16),
    }


def _prep_percore(bc_core):
    """bc_core: [BL, NIV, 4, IN] fp32 -> x0t [IN, BL] f16, dxh [128, NS*NIV*INP] f16."""
    x0t = bc_core[:, 0, 0, :].T.astype(np.float16)
    c1 = bc_core[:, :, 1, :]  # [BL, NIV, IN]
    c2 = bc_core[:, :, 2, :]
    c3 = bc_core[:, :, 3, :]
    dxh = np.zeros((128, NIV * NS * INP), np.float32)
    for iv in range(NIV):
        for si in range(NS):
            s = si * 0.25
            dX = c1[:, iv] + (2.0 * s) * c2[:, iv] + (3.0 * s * s) * c3[:, iv]
            col = (iv * NS + si) * INP
            dxh[0:BL, col : col + IN] = dX
            dxh[BL:128, col : col + IN] = dX
    return x0t, dxh.astype(np.float16)


def build_nc(steps=STEPS):
    """Build the single-core Bass program (same program on all 8 cores)."""
    from contextlib import ExitStack

    import concourse.bass as bass
    import concourse.mybir as mybir
    from concourse import bacc, tile

    f16 = mybir.dt.float16
    f32 = mybir.dt.float32
    AF = mybir.ActivationFunctionType
    OP = mybir.AluOpType

    nc = bacc.Bacc("TRN2", target_bir_lowering=False, debug=False)

    dram = {}
    ins_spec = [
        ("x0t", [IN, BL], f16),
        ("dxh", [128, NIV * NS * INP], f16),
        ("w0", [IN, HID], f16),
        ("b0c", [HID, 1], f32),
        ("wi", [HID, HH], f16),
        ("wha", [128, NL * HH], f16),
        ("whb", [HH - 128, NL * HH], f16),
        ("bias_a", [128, 1 + NL], f32),
        ("bias_b", [HH - 128, 1 + NL], f32),
        ("woa", [128, 2 * HALF_COLS], f16),
        ("wob", [HH - 128 + 1, 2 * HALF_COLS], f16),
        ("wc1", [HID, HID], f16),
        ("bc1c", [HID, 1], f32),
        ("wc2", [HID, NCLS], f16),
        ("bc2c", [NCLS, 1], f32),
        ("eperm", [128, 128], f16),
        ("hhB_init", [HH - 128 + 1, BL], f16),
    ]
    for name, shape, dt in ins_spec:
        dram[name] = nc.dram_tensor(name, shape, dt, kind="ExternalInput")
    out_dram = nc.dram_tensor("pred_t", [NCLS, BL], f32, kind="ExternalOutput")

    with tile.TileContext(nc) as tc:
        with ExitStack() as ctx:
            const = ctx.enter_context(tc.tile_pool(name="const", bufs=1))
            work = ctx.enter_context(tc.tile_pool(name="work", bufs=3))
            ty_pool = ctx.enter_context(tc.tile_pool(name="ty", bufs=3))
            pr_pool = ctx.enter_context(tc.tile_pool(name="pr", bufs=3))
            kt_pool = ctx.enter_context(tc.tile_pool(name="kt", bufs=2))
            ps_h = ctx.enter_context(
                tc.tile_pool(name="ps_h", bufs=2, space=bass.MemorySpace.PSUM)
            )
            ps_hb = ctx.enter_context(
                tc.tile_pool(name="ps_hb", bufs=2, space=bass.MemorySpace.PSUM)
            )
            ps_y = ctx.enter_context(
                tc.tile_pool(name="ps_y", bufs=1, space=bass.MemorySpace.PSUM)
            )
            ps_k = ctx.enter_context(
                tc.tile_pool(name="ps_k", bufs=1, space=bass.MemorySpace.PSUM)
            )

            # ---- load constants/weights into SBUF --------------------------
            # two HWDGE queues (SP + ACT) in parallel, earliest-needed
            # tensors first on each; big einsum/W_out tensors next,
            # classifier last
            dma_sp = ["w0", "x0t", "b0c", "wi", "bias_a", "eperm", "woa",
                      "dxh", "wc1", "bc1c"]
            dma_act = ["wha", "whb", "bias_b", "hhB_init", "wob",
                       "wc2", "bc2c"]
            shapes = {name: (shape, dt) for name, shape, dt in ins_spec}
            sb = {}
            for names, eng in ((dma_sp, nc.sync), (dma_act, nc.scalar)):
                for name in names:
                    shape, dt = shapes[name]
                    t = const.tile(shape, dt, tag=name)
                    eng.dma_start(t[:], dram[name][:])
                    sb[name] = t

            # persistent state tiles (hhB arrives with its ones row preset)
            hhB = sb["hhB_init"]
            zfmA = const.tile([HID, BL], f32, tag="zfmA")   # master z (feature-major)
            zfmB = const.tile([HID, BL], f32, tag="zfmB")
            zacc = const.tile([HID, BL], f32, tag="zacc")

            # psum y region tiles (persistent; serial stages reuse them)
            # one full 2KB bank each so every tile starts bank-aligned
            yR = [
                ps_y.tile([128, 512], f32, tag=f"yR{rt}", name=f"yR{rt}")
                for rt in range(3)
            ]

            # ---- initial state z0 = X0 @ W0 + b0 (feature-major) -----------
            z0p = ps_h.tile([HID, BL], f32, tag="hA")
            nc.tensor.matmul(z0p[:], sb["w0"][:], sb["x0t"][:])
            zT = work.tile([HID, BL], f16, tag="zT")
            nc.vector.tensor_scalar(zT[:], z0p[:], sb["b0c"][:], None, OP.add)
            zfm = zfmA
            zfm_nxt = zfmB
            nc.vector.tensor_scalar(zfm[:], z0p[:], sb["b0c"][:], None, OP.add)

            # RK4 coefficients
            acc_w = [HSTEP / 6.0, HSTEP / 3.0, HSTEP / 3.0, HSTEP / 6.0]
            inp_w = [0.5 * HSTEP, 0.5 * HSTEP, HSTEP, None]

            # off-critical-path accumulator updates are deferred into the
            # next stage's MLP phase so they never delay the zT handoff
            pending_acc = []

            def flush_acc():
                while pending_acc:
                    out_t, in0, scal, in1 = pending_acc.pop(0)
                    nc.vector.scalar_tensor_tensor(
                        out_t[:], in0[:], scal, in1[:], OP.mult, OP.add
                    )

            # ---- time stepping --------------------------------------------
            for step in range(steps):
                iv, sub = step // N_SUB, step % N_SUB
                for stg in range(4):
                    sidx = 2 * sub + (0 if stg == 0 else (1 if stg < 3 else 2))
                    dxcol = (iv * NS + sidx) * INP

                    # -- small MLP: W_in then NL hidden layers (feature-major)
                    hA = None
                    hB = None
                    for layer in range(1 + NL):
                        if layer == 0:
                            wa_l = sb["wi"][:]
                            wb_l = None
                        else:
                            c0 = (layer - 1) * HH
                            wa_l = sb["wha"][:, c0 : c0 + HH]
                            wb_l = sb["whb"][:, c0 : c0 + HH]
                        pA = ps_h.tile([128, BL], f32, tag="hA")
                        pB = ps_hb.tile([HH - 128, BL], f32, tag="hB")
                        if layer == 0:
                            nc.tensor.matmul(pA[:], wa_l[:, 0:128], zT[:])
                            nc.tensor.matmul(pB[:], wa_l[:, 128:HH], zT[:])
                        else:
                            # pA pair first: its relu gates the next layer,
                            # while the pB pair streams during that relu
                            nc.tensor.matmul(
                                pA[:], wa_l[:, 0:128], hA[:], start=True, stop=False
                            )
                            nc.tensor.matmul(
                                pA[:], wb_l[:, 0:128], hB[:], start=False, stop=True
                            )
                            nc.tensor.matmul(
                                pB[:], wa_l[:, 128:HH], hA[:], start=True, stop=False
                            )
                            nc.tensor.matmul(
                                pB[:], wb_l[:, 128:HH], hB[:], start=False, stop=True
                            )
                        last = layer == NL
                        nhA = work.tile([128, BL], f16, tag="hA_sb")
                        nhB = hhB[0 : HH - 128, :] if last else work.tile(
                            [HH - 128, BL], f16, tag="hB_sb"
                        )
                        ba = sb["bias_a"][:, layer : layer + 1]
                        bb = sb["bias_b"][:, layer : layer + 1]
                        # A on DVE (lower PSUM->SBUF latency than ACT), B on
                        # ACT: the two bias+relu ops run concurrently
                        nc.vector.tensor_scalar(nhA[:], pA[:], ba, 0.0, OP.add, OP.max)
                        nc.scalar.activation(nhB[:], pB[:], AF.Relu, bias=bb)
                        hA, hB = nhA, (hhB[0 : HH - 128 + 1, :] if last else nhB)
                        if layer == 0:
                            flush_acc()

                    # -- W_out: y[p = half*64+b, (h_local, i)]  (batch-major)
                    # region-major so each region completes early and the
                    # tanh/einsum chain starts while later regions stream
                    for rt, (h0, hc, bk, bc) in enumerate(REGIONS):
                        for kc in range(2):
                            lhs = hA[:] if kc == 0 else hhB[:]
                            rhs_t = sb["woa"] if kc == 0 else sb["wob"]
                            for half in range(2):
                                cols = half * HALF_COLS + h0 * INP
                                # lo/hi halves accumulate in disjoint
                                # partition rows of one bank; the sim's group
                                # guard is partition-blind, so skip it.
                                nc.tensor.matmul(
                                    yR[bk][half * 64 : half * 64 + 64, bc : bc + hc * INP],
                                    lhs,
                                    rhs_t[:, cols : cols + hc * INP],
                                    start=(kc == 0),
                                    stop=(kc == 1),
                                    skip_group_check=True,
                                )

                    # -- tanh -> multiply by dX -> segmented reduce over i
                    # (pad col 41 skipped on DVE; k_t fp16 for 2x multiply)
                    k_t = kt_pool.tile([128, 32], f16, tag="k")
                    for rt, (h0, hc, bk, bc) in enumerate(REGIONS):
                        # tanh skips the pad column via a 3D strided read
                        ty = ty_pool.tile([128, hc * IN], f16, tag=f"ty{rt}")
                        yv = (
                            yR[bk][:, bc : bc + hc * INP]
                            .rearrange("p (h i) -> p h i", i=INP)[:, :, 0:IN]
                        )
                        tyv = ty[:].rearrange("p (h i) -> p h i", i=IN)
                        nc.scalar.activation(tyv, yv, AF.Tanh)
                        pr = pr_pool.tile([128, hc * IN], f16, tag=f"pr{rt}")
                        dxv = (
                            sb["dxh"][:, dxcol : dxcol + IN]
                            .unsqueeze(1)
                            .broadcast_to((128, hc, IN))
                        )
                        prv = pr[:].rearrange("p (h i) -> p h i", i=IN)
                        nc.vector.tensor_tensor(prv, tyv, dxv, OP.mult)
                        with nc.allow_low_precision(reason="k reduce fp16 ok"):
                            nc.vector.tensor_reduce(
                                k_t[:, h0 : h0 + hc], prv, mybir.AxisListType.X, OP.add
                            )

                    # -- transpose k to feature-major in region groups so the
                    # z update is a single Pool op with no cast afterwards
                    kfm = ps_k.tile([HID, BL], f32, tag="kfm")
                    for h0, hc, _ in KT_GROUPS:
                        nc.tensor.matmul(
                            kfm[h0 : h0 + hc, :],
                            k_t[:, h0 : h0 + hc],
                            sb["eperm"][:, 0:64],
                            skip_group_check=True,
                        )
                        nc.tensor.matmul(
                            kfm[32 + h0 : 32 + h0 + hc, :],
                            k_t[:, h0 : h0 + hc],
                            sb["eperm"][:, 64:128],
                            skip_group_check=True,
                        )

                    # -- z updates, all feature-major.  zT (gates next MLP)
                    # now; the accumulator update is deferred into the next
                    # stage's MLP phase (flush_acc)
                    zT = work.tile([HID, BL], f16, tag="zT")
                    if stg < 3:
                        nc.vector.scalar_tensor_tensor(
                            zT[:], kfm[:], inp_w[stg], zfm[:], OP.mult, OP.add
                        )
                        pending_acc.append(
                            (zacc, kfm, acc_w[stg], zfm if stg == 0 else zacc)
                        )
                    else:
                        nc.vector.scalar_tensor_tensor(
                            zT[:], kfm[:], acc_w[3], zacc[:], OP.mult, OP.add
                        )
                        pending_acc.append((zfm_nxt, kfm, acc_w[3], zacc))
                        zfm, zfm_nxt = zfm_nxt, zfm

            # ---- classifier on final state --------------------------------
            pending_acc.clear()  # final master-z write is never read
            c1p = ps_h.tile([HID, BL], f32, tag="hA")
            nc.tensor.matmul(c1p[:], sb["wc1"][:], zT[:])
            c1 = work.tile([HID, BL], f16, tag="c1")
            nc.vector.tensor_scalar(c1[:], c1p[:], sb["bc1c"][:], 0.0, OP.add, OP.max)
            c2p = ps_hb.tile([NCLS, BL], f32, tag="hB")
            nc.tensor.matmul(c2p[:], sb["wc2"][:], c1[:])
            pred = work.tile([NCLS, BL], f32, tag="pred")
            nc.vector.tensor_scalar(pred[:], c2p[:], sb["bc2c"][:], None, OP.add)
            nc.sync.dma_start(out_dram[:], pred[:])

    nc.compile()
    return nc


def make_in_maps(inputs):
    shared = _prep_shared(
        inputs["W0"], inputs["b0"], inputs["W_in"], inputs["b_in"],
        inputs["W_h"], inputs["b_h"], inputs["W_out"], inputs["b_out"],
        inputs["Wc1"], inputs["bc1"], inputs["Wc2"], inputs["bc2"],
    )
    bc = np.asarray(inputs["batch_coeffs"], np.float32)
    in_maps = []
    for c in range(NCORES):
        x0t, dxh = _prep_percore(bc[c * BL : (c + 1) * BL])
        in_maps.append({**shared, "x0t": x0t, "dxh": dxh})
    return in_maps


_CACHED = {}


def kernel(**inputs):
    from concourse.bass_utils import run_bass_kernel_spmd

    if "nc" not in _CACHED:
        _CACHED["nc"] = build_nc()
    nc = _CACHED["nc"]
    in_maps = make_in_maps(inputs)
    res = run_bass_kernel_spmd(
        nc, in_maps, core_ids=list(range(NCORES)),
        trace=bool(int(os.environ.get("NCDE_TRACE", "0"))),
    )
    _CACHED["last_result"] = res
    out = np.zeros((B, NCLS), np.float32)
    for c in range(NCORES):
        out[c * BL : (c + 1) * BL, :] = res.results[c]["pred_t"].T
    return out


# revision 52
# speedup vs baseline: 1.0426x; 1.0144x over previous
"""Neural CDE forward pass on 8 Trainium2 NeuronCores (pure data parallel).

B=512 batch is sharded 64 per core.  Per core, the entire 30-step RK4
integration (120 vector-field evals) runs out of SBUF:

  - small MLP layers feature-major: lhsT = weight chunk (stationary),
    rhs = activation [feat, 64]; bias+relu on Pool (A half) and DVE
    (B half) so ACT stays free for tanh.
  - W_out layer batch-major, split-h: y PSUM [128 = 2 h-halves x 64 batch,
    cols = (h_local, i_pad)] so tanh/mul/reduce use all 128 lanes.
    b_out enters via a ones-row appended to the stationary activation.
  - einsum g[b,h] = sum_i tanh(y)[b,h,i] * dX[b,i]: DVE multiply with a
    broadcast dX tile (fp16) + segmented fp16 tensor_reduce over i.
  - k is transposed to feature-major per region-group with small PE
    matmuls through a stacked identity, so the z state stays
    feature-major and never needs a post-update transpose or cast.
  - dX for the 75 distinct (interval, s) points is computed on host and
    DMA'd once.
"""

import os
import numpy as np

B, T, IN, HID, HH, NCLS = 512, 16, 41, 64, 150, 4
N_SUB, NL = 2, 3
NCORES = 8
BL = B // NCORES            # 64 per-core batch
INP = IN + 1                # 42: i padded for even segments
NIV = T - 1                 # 15 intervals
NS = 5                      # distinct s values per interval
HSTEP = 1.0 / N_SUB         # 0.5
HALF_COLS = 32 * INP        # 1344 cols per h-half
# (h0, hcount, bank, bank_col) splits of the 32 h per half; each region's
# cols (hcount*42) must fit one 2KB PSUM bank (<=512 fp32) since a matmul
# cannot cross banks.  First region small so the tanh/mul/reduce chain
# starts early; last region tiny so the final reduce drains fast.  Regions
# 0 and 3 share PSUM bank 0 at different column offsets.
REGIONS = [(0, 11, 0, 0), (11, 11, 1, 0), (22, 10, 2, 0)]
# k-transpose groups (h0, hc): matmul out base partition must be 0/32/64,
# so a single [0,32) group (bases 0 and 32) is the only legal split
KT_GROUPS = [(0, 32, 2)]  # (h0, hc, after_region_idx)
STEPS = int(os.environ.get("NCDE_STEPS", NIV * N_SUB))  # debug knob


def _prep_shared(W0, b0, W_in, b_in, W_h, b_h, W_out, b_out, Wc1, bc1, Wc2, bc2):
    f16 = np.float16
    f32 = np.float32
    wha = np.concatenate([W_h[i][0:128, :] for i in range(NL)], axis=1)
    whb = np.concatenate([W_h[i][128:HH, :] for i in range(NL)], axis=1)
    bias_a = np.stack([b_in[0:128]] + [b_h[i][0:128] for i in range(NL)], axis=1)
    bias_b = np.stack([b_in[128:HH]] + [b_h[i][128:HH] for i in range(NL)], axis=1)
    R = W_out.reshape(HH, HID, IN)
    Rp = np.zeros((HH, HID, INP), np.float32)
    Rp[:, :, :IN] = R
    W2 = np.concatenate(
        [Rp[:, 0:32, :].reshape(HH, HALF_COLS), Rp[:, 32:64, :].reshape(HH, HALF_COLS)],
        axis=1,
    )
    bo = np.zeros((HID, INP), np.float32)
    bo[:, :IN] = b_out.reshape(HID, IN)
    bo2 = np.concatenate([bo[0:32].reshape(-1), bo[32:64].reshape(-1)])
    i64 = np.eye(64, dtype=np.float32)
    return {
        "w0": W0.astype(f16),
        "b0c": b0.reshape(HID, 1).astype(f32),
        "wi": W_in.astype(f16),
        "wha": wha.astype(f16),
        "whb": whb.astype(f16),
        "bias_a": bias_a.astype(f32),
        "bias_b": bias_b.astype(f32),
        "woa": W2[0:128].astype(f16),
        "wob": np.vstack([W2[128:HH], bo2[None]]).astype(f16),
        "wc1": Wc1.astype(f16),
        "bc1c": bc1.reshape(HID, 1).astype(f32),
        "wc2": Wc2.astype(f16),
        "bc2c": bc2.reshape(NCLS, 1).astype(f32),
        "eperm": np.vstack(
            [np.hstack([i64, 0 * i64]), np.hstack([0 * i64, i64])]
        ).astype(f16),
        "hhB_init": np.vstack(
            [np.zeros((HH - 128, BL), np.float32), np.ones((1, BL), np.float32)]
        ).astype(f16),
    }


def _prep_percore(bc_core):
    """bc_core: [BL, NIV, 4, IN] fp32 -> x0t [IN, BL] f16, dxh [128, NS*NIV*INP] f16."""
    x0t = bc_core[:, 0, 0, :].T.astype(np.float16)
    c1 = bc_core[:, :, 1, :]  # [BL, NIV, IN]
    c2 = bc_core[:, :, 2, :]
    c3 = bc_core[:, :, 3, :]
    dxh = np.zeros((128, NIV * NS * INP), np.float32)
    for iv in range(NIV):
        for si in range(NS):
            s = si * 0.25
            dX = c1[:, iv] + (2.0 * s) * c2[:, iv] + (3.0 * s * s) * c3[:, iv]
            col = (iv * NS + si) * INP
            dxh[0:BL, col : col + IN] = dX
            dxh[BL:128, col : col + IN] = dX
    return x0t, dxh.astype(np.float16)


def build_nc(steps=STEPS):
    """Build the single-core Bass program (same program on all 8 cores)."""
    from contextlib import ExitStack

    import concourse.bass as bass
    import concourse.mybir as mybir
    from concourse import bacc, tile

    f16 = mybir.dt.float16
    f32 = mybir.dt.float32
    AF = mybir.ActivationFunctionType
    OP = mybir.AluOpType

    nc = bacc.Bacc("TRN2", target_bir_lowering=False, debug=False)

    dram = {}
    ins_spec = [
        ("x0t", [IN, BL], f16),
        ("dxh", [128, NIV * NS * INP], f16),
        ("w0", [IN, HID], f16),
        ("b0c", [HID, 1], f32),
        ("wi", [HID, HH], f16),
        ("wha", [128, NL * HH], f16),
        ("whb", [HH - 128, NL * HH], f16),
        ("bias_a", [128, 1 + NL], f32),
        ("bias_b", [HH - 128, 1 + NL], f32),
        ("woa", [128, 2 * HALF_COLS], f16),
        ("wob", [HH - 128 + 1, 2 * HALF_COLS], f16),
        ("wc1", [HID, HID], f16),
        ("bc1c", [HID, 1], f32),
        ("wc2", [HID, NCLS], f16),
        ("bc2c", [NCLS, 1], f32),
        ("eperm", [128, 128], f16),
        ("hhB_init", [HH - 128 + 1, BL], f16),
    ]
    for name, shape, dt in ins_spec:
        dram[name] = nc.dram_tensor(name, shape, dt, kind="ExternalInput")
    out_dram = nc.dram_tensor("pred_t", [NCLS, BL], f32, kind="ExternalOutput")

    with tile.TileContext(nc) as tc:
        with ExitStack() as ctx:
            const = ctx.enter_context(tc.tile_pool(name="const", bufs=1))
            work = ctx.enter_context(tc.tile_pool(name="work", bufs=4))
            ty_pool = ctx.enter_context(tc.tile_pool(name="ty", bufs=3))
            pr_pool = ctx.enter_context(tc.tile_pool(name="pr", bufs=3))
            kt_pool = ctx.enter_context(tc.tile_pool(name="kt", bufs=2))
            ps_h = ctx.enter_context(
                tc.tile_pool(name="ps_h", bufs=2, space=bass.MemorySpace.PSUM)
            )
            ps_hb = ctx.enter_context(
                tc.tile_pool(name="ps_hb", bufs=2, space=bass.MemorySpace.PSUM)
            )
            ps_y = ctx.enter_context(
                tc.tile_pool(name="ps_y", bufs=1, space=bass.MemorySpace.PSUM)
            )
            ps_k = ctx.enter_context(
                tc.tile_pool(name="ps_k", bufs=1, space=bass.MemorySpace.PSUM)
            )

            # ---- load constants/weights into SBUF --------------------------
            # two HWDGE queues (SP + ACT) in parallel, earliest-needed
            # tensors first on each; big einsum/W_out tensors next,
            # classifier last
            dma_sp = ["w0", "x0t", "b0c", "wi", "bias_a", "eperm", "woa",
                      "dxh", "wc1", "bc1c"]
            dma_act = ["wha", "whb", "bias_b", "hhB_init", "wob",
                       "wc2", "bc2c"]
            shapes = {name: (shape, dt) for name, shape, dt in ins_spec}
            sb = {}
            for names, eng in ((dma_sp, nc.sync), (dma_act, nc.scalar)):
                for name in names:
                    shape, dt = shapes[name]
                    t = const.tile(shape, dt, tag=name)
                    eng.dma_start(t[:], dram[name][:])
                    sb[name] = t

            # persistent state tiles (hhB arrives with its ones row preset)
            hhB = sb["hhB_init"]
            zfmA = const.tile([HID, BL], f32, tag="zfmA")   # master z (feature-major)
            zfmB = const.tile([HID, BL], f32, tag="zfmB")
            zacc = const.tile([HID, BL], f32, tag="zacc")

            # psum y region tiles (persistent; serial stages reuse them)
            # one full 2KB bank each so every tile starts bank-aligned
            yR = [
                ps_y.tile([128, 512], f32, tag=f"yR{rt}", name=f"yR{rt}")
                for rt in range(3)
            ]

            # ---- initial state z0 = X0 @ W0 + b0 (feature-major) -----------
            z0p = ps_h.tile([HID, BL], f32, tag="hA")
            nc.tensor.matmul(z0p[:], sb["w0"][:], sb["x0t"][:])
            zT = work.tile([HID, BL], f16, tag="zT")
            nc.vector.tensor_scalar(zT[:], z0p[:], sb["b0c"][:], None, OP.add)
            zfm = zfmA
            zfm_nxt = zfmB
            nc.vector.tensor_scalar(zfm[:], z0p[:], sb["b0c"][:], None, OP.add)

            # RK4 coefficients
            acc_w = [HSTEP / 6.0, HSTEP / 3.0, HSTEP / 3.0, HSTEP / 6.0]
            inp_w = [0.5 * HSTEP, 0.5 * HSTEP, HSTEP, None]

            # off-critical-path accumulator updates are deferred into the
            # next stage's MLP phase so they never delay the zT handoff
            pending_acc = []

            def flush_acc():
                while pending_acc:
                    out_t, in0, scal, in1 = pending_acc.pop(0)
                    nc.vector.scalar_tensor_tensor(
                        out_t[:], in0[:], scal, in1[:], OP.mult, OP.add
                    )

            # ---- time stepping --------------------------------------------
            for step in range(steps):
                iv, sub = step // N_SUB, step % N_SUB
                for stg in range(4):
                    sidx = 2 * sub + (0 if stg == 0 else (1 if stg < 3 else 2))
                    dxcol = (iv * NS + sidx) * INP

                    # -- small MLP: W_in then NL hidden layers (feature-major)
                    hA = None
                    hB = None
                    for layer in range(1 + NL):
                        if layer == 0:
                            wa_l = sb["wi"][:]
                            wb_l = None
                        else:
                            c0 = (layer - 1) * HH
                            wa_l = sb["wha"][:, c0 : c0 + HH]
                            wb_l = sb["whb"][:, c0 : c0 + HH]
                        pA = ps_h.tile([128, BL], f32, tag="hA")
                        pB = ps_hb.tile([HH - 128, BL], f32, tag="hB")
                        if layer == 0:
                            nc.tensor.matmul(pA[:], wa_l[:, 0:128], zT[:])
                            nc.tensor.matmul(pB[:], wa_l[:, 128:HH], zT[:])
                        else:
                            # pA pair first: its relu gates the next layer,
                            # while the pB pair streams during that relu
                            nc.tensor.matmul(
                                pA[:], wa_l[:, 0:128], hA[:], start=True, stop=False
                            )
                            nc.tensor.matmul(
                                pA[:], wb_l[:, 0:128], hB[:], start=False, stop=True
                            )
                            nc.tensor.matmul(
                                pB[:], wa_l[:, 128:HH], hA[:], start=True, stop=False
                            )
                            nc.tensor.matmul(
                                pB[:], wb_l[:, 128:HH], hB[:], start=False, stop=True
                            )
                        last = layer == NL
                        nhA = work.tile([128, BL], f16, tag="hA_sb")
                        nhB = hhB[0 : HH - 128, :] if last else work.tile(
                            [HH - 128, BL], f16, tag="hB_sb"
                        )
                        ba = sb["bias_a"][:, layer : layer + 1]
                        bb = sb["bias_b"][:, layer : layer + 1]
                        # A on DVE (lower PSUM->SBUF latency than ACT), B on
                        # ACT: the two bias+relu ops run concurrently
                        nc.vector.tensor_scalar(nhA[:], pA[:], ba, 0.0, OP.add, OP.max)
                        nc.scalar.activation(nhB[:], pB[:], AF.Relu, bias=bb)
                        hA, hB = nhA, (hhB[0 : HH - 128 + 1, :] if last else nhB)
                        if layer == 0:
                            flush_acc()

                    # -- W_out: y[p = half*64+b, (h_local, i)]  (batch-major)
                    # region-major so each region completes early and the
                    # tanh/einsum chain starts while later regions stream
                    for rt, (h0, hc, bk, bc) in enumerate(REGIONS):
                        for kc in range(2):
                            lhs = hA[:] if kc == 0 else hhB[:]
                            rhs_t = sb["woa"] if kc == 0 else sb["wob"]
                            for half in range(2):
                                cols = half * HALF_COLS + h0 * INP
                                # lo/hi halves accumulate in disjoint
                                # partition rows of one bank; the sim's group
                                # guard is partition-blind, so skip it.
                                nc.tensor.matmul(
                                    yR[bk][half * 64 : half * 64 + 64, bc : bc + hc * INP],
                                    lhs,
                                    rhs_t[:, cols : cols + hc * INP],
                                    start=(kc == 0),
                                    stop=(kc == 1),
                                    skip_group_check=True,
                                )

                    # -- tanh -> multiply by dX -> segmented reduce over i
                    # (pad col 41 skipped on DVE; k_t fp16 for 2x multiply)
                    k_t = kt_pool.tile([128, 32], f16, tag="k")
                    for rt, (h0, hc, bk, bc) in enumerate(REGIONS):
                        # tanh skips the pad column via a 3D strided read
                        ty = ty_pool.tile([128, hc * IN], f16, tag=f"ty{rt}")
                        yv = (
                            yR[bk][:, bc : bc + hc * INP]
                            .rearrange("p (h i) -> p h i", i=INP)[:, :, 0:IN]
                        )
                        tyv = ty[:].rearrange("p (h i) -> p h i", i=IN)
                        nc.scalar.activation(tyv, yv, AF.Tanh)
                        pr = pr_pool.tile([128, hc * IN], f16, tag=f"pr{rt}")
                        dxv = (
                            sb["dxh"][:, dxcol : dxcol + IN]
                            .unsqueeze(1)
                            .broadcast_to((128, hc, IN))
                        )
                        prv = pr[:].rearrange("p (h i) -> p h i", i=IN)
                        nc.vector.tensor_tensor(prv, tyv, dxv, OP.mult)
                        with nc.allow_low_precision(reason="k reduce fp16 ok"):
                            nc.vector.tensor_reduce(
                                k_t[:, h0 : h0 + hc], prv, mybir.AxisListType.X, OP.add
                            )

                    # -- transpose k to feature-major in region groups so the
                    # z update is a single Pool op with no cast afterwards
                    kfm = ps_k.tile([HID, BL], f32, tag="kfm")
                    for h0, hc, _ in KT_GROUPS:
                        nc.tensor.matmul(
                            kfm[h0 : h0 + hc, :],
                            k_t[:, h0 : h0 + hc],
                            sb["eperm"][:, 0:64],
                            skip_group_check=True,
                        )
                        nc.tensor.matmul(
                            kfm[32 + h0 : 32 + h0 + hc, :],
                            k_t[:, h0 : h0 + hc],
                            sb["eperm"][:, 64:128],
                            skip_group_check=True,
                        )

                    # -- z updates, all feature-major.  zT (gates next MLP)
                    # now; the accumulator update is deferred into the next
                    # stage's MLP phase (flush_acc)
                    zT = work.tile([HID, BL], f16, tag="zT")
                    if stg < 3:
                        nc.vector.scalar_tensor_tensor(
                            zT[:], kfm[:], inp_w[stg], zfm[:], OP.mult, OP.add
                        )
                        pending_acc.append(
                            (zacc, kfm, acc_w[stg], zfm if stg == 0 else zacc)
                        )
                    else:
                        nc.vector.scalar_tensor_tensor(
                            zT[:], kfm[:], acc_w[3], zacc[:], OP.mult, OP.add
                        )
                        pending_acc.append((zfm_nxt, kfm, acc_w[3], zacc))
                        zfm, zfm_nxt = zfm_nxt, zfm

            # ---- classifier on final state --------------------------------
            pending_acc.clear()  # final master-z write is never read
            c1p = ps_h.tile([HID, BL], f32, tag="hA")
            nc.tensor.matmul(c1p[:], sb["wc1"][:], zT[:])
            c1 = work.tile([HID, BL], f16, tag="c1")
            nc.vector.tensor_scalar(c1[:], c1p[:], sb["bc1c"][:], 0.0, OP.add, OP.max)
            c2p = ps_hb.tile([NCLS, BL], f32, tag="hB")
            nc.tensor.matmul(c2p[:], sb["wc2"][:], c1[:])
            pred = work.tile([NCLS, BL], f32, tag="pred")
            nc.vector.tensor_scalar(pred[:], c2p[:], sb["bc2c"][:], None, OP.add)
            nc.sync.dma_start(out_dram[:], pred[:])

    nc.compile()
    return nc


def make_in_maps(inputs):
    shared = _prep_shared(
        inputs["W0"], inputs["b0"], inputs["W_in"], inputs["b_in"],
        inputs["W_h"], inputs["b_h"], inputs["W_out"], inputs["b_out"],
        inputs["Wc1"], inputs["bc1"], inputs["Wc2"], inputs["bc2"],
    )
    bc = np.asarray(inputs["batch_coeffs"], np.float32)
    in_maps = []
    for c in range(NCORES):
        x0t, dxh = _prep_percore(bc[c * BL : (c + 1) * BL])
        in_maps.append({**shared, "x0t": x0t, "dxh": dxh})
    return in_maps


_CACHED = {}


def kernel(**inputs):
    from concourse.bass_utils import run_bass_kernel_spmd

    if "nc" not in _CACHED:
        _CACHED["nc"] = build_nc()
    nc = _CACHED["nc"]
    in_maps = make_in_maps(inputs)
    res = run_bass_kernel_spmd(
        nc, in_maps, core_ids=list(range(NCORES)),
        trace=bool(int(os.environ.get("NCDE_TRACE", "0"))),
    )
    _CACHED["last_result"] = res
    out = np.zeros((B, NCLS), np.float32)
    for c in range(NCORES):
        out[c * BL : (c + 1) * BL, :] = res.results[c]["pred_t"].T
    return out
